# revision 37
# baseline (speedup 1.0000x reference)
"""Multi-head attention (B=8, N=1024, D=512, H=8) on 8 TRN2 NeuronCores.

Sharding: pure batch-parallel — core i computes batch i end-to-end, no
collectives. Host-side prep per batch: gather valid keys (mask) into a
contiguous buffer padded to NKV=640, pre-transpose x, and build small
per-partition bias/validity tables. Device pipeline (all matmuls fp32r):
  k^T/q^T/v projections -> scores s^T[k,q] per head -> exp on ACT with the
  key-padding mask folded into the per-partition activation bias ->
  attn@v with an augmented ones-column producing the softmax denominator
  in row 64 -> fast reciprocal + Pool partition-broadcast -> normalize ->
  out-projection.

Math shortcuts: bk is dropped (constant-in-key terms cancel in softmax);
bv is folded into the output bias on the host (bob' = bo + bv @ wo since
normalized attention rows sum to 1).
"""

import sys

import numpy as np

sys.path.insert(0, "/opt/trn_rl_repo")

B, N, D, H = 8, 1024, 512, 8
HD = D // H            # 64
SCALE = HD ** -0.5     # 0.125
NKV = 640              # padded valid-key count (5 chunks of 128)
KC = NKV // 128        # 5
DC = D // 128          # 4
PAD_BIAS = -30000.0    # exp(PAD_BIAS + s*SCALE) == 0.0 exactly in fp32

_prog_cache = {}


def _build_program():
    import concourse.bacc as bacc
    import concourse.tile as tile
    from concourse import mybir

    dt = mybir.dt
    f32 = dt.float32
    f32r = dt.float32r
    AF = mybir.ActivationFunctionType

    def r(ap):
        return ap.bitcast(f32r)

    nc = bacc.Bacc("TRN2", target_bir_lowering=False, debug=False)

    xT_d = nc.dram_tensor("xT", [D, N], f32, kind="ExternalInput").ap()
    xkT_d = nc.dram_tensor("xkT", [D, NKV], f32, kind="ExternalInput").ap()
    wq_d = nc.dram_tensor("wq", [D, D], f32, kind="ExternalInput").ap()
    wk_d = nc.dram_tensor("wk", [D, D], f32, kind="ExternalInput").ap()
    wv_d = nc.dram_tensor("wv", [D, D], f32, kind="ExternalInput").ap()
    wo_d = nc.dram_tensor("wo", [D, D], f32, kind="ExternalInput").ap()
    bq_d = nc.dram_tensor("bq", [D, 1], f32, kind="ExternalInput").ap()
    bob_d = nc.dram_tensor("bob", [128, D], f32, kind="ExternalInput").ap()
    expb_d = nc.dram_tensor("expb", [128, KC], f32, kind="ExternalInput").ap()
    onesv_d = nc.dram_tensor("onesv", [128, H, 1], f32, kind="ExternalInput").ap()
    y_d = nc.dram_tensor("y", [N, D], f32, kind="ExternalOutput").ap()

    with tile.TileContext(nc) as tc, \
         nc.allow_low_precision(reason="float32r == fp32 rounded for PE fast path"):
        with tc.tile_pool(name="const", bufs=1) as cpool:
            # Persistent result tiles (vaug gets its ones column via DMA below)
            qT_t = [cpool.tile([128, N], f32, name=f"qT_t{c}") for c in range(DC)]
            kT_t = [cpool.tile([128, NKV], f32, name=f"kT_t{c}") for c in range(DC)]
            vaug_t = [cpool.tile([128, H, HD + 1], f32, name=f"vaug_t{c}")
                      for c in range(KC)]
            aoT_t = [cpool.tile([128, N], f32, name=f"aoT_t{c}") for c in range(DC)]

            # --- DMA issue order = priority order (queues drain round-robin).
            # k-projection inputs first (PE starts on them), tiny tables,
            # then q inputs, v, and wo/bob last. Big loads split into
            # partition halves so they spread across two queues.
            def load_half(t, dram_slice):
                nc.sync.dma_start(r(t[0:64, :]), r(dram_slice[0:64, :]))
                nc.sync.dma_start(r(t[64:128, :]), r(dram_slice[64:128, :]))

            def alloc_w(name):
                return [cpool.tile([128, D], f32, name=f"{name}{c}")
                        for c in range(DC)]

            wk_t, wq_t, wv_t, wo_t = (alloc_w(n) for n in
                                      ("wk_t", "wq_t", "wv_t", "wo_t"))
            xkT_t = [cpool.tile([128, NKV], f32, name=f"xkT_t{c}")
                     for c in range(DC)]
            xT_t = [cpool.tile([128, N], f32, name=f"xT_t{c}")
                    for c in range(DC)]

            for c in range(DC):
                load_half(wk_t[c], wk_d[128 * c:128 * (c + 1), :])
                load_half(xkT_t[c], xkT_d[128 * c:128 * (c + 1), :])
            bq_t = cpool.tile([128, DC], f32, name="bq_t")
            for c in range(DC):
                nc.sync.dma_start(bq_t[:, c:c + 1], bq_d[128 * c:128 * (c + 1), :])
            expb_t = cpool.tile([128, KC], f32, name="expb_t")
            nc.sync.dma_start(expb_t[:], expb_d[:, :])
            for c in range(KC):
                nc.sync.dma_start(r(vaug_t[c][:, :, HD:HD + 1]), r(onesv_d[:]))
            for c in range(DC):
                load_half(wq_t[c], wq_d[128 * c:128 * (c + 1), :])
                load_half(xT_t[c], xT_d[128 * c:128 * (c + 1), :])
            for c in range(DC):
                load_half(wv_t[c], wv_d[128 * c:128 * (c + 1), :])
            for c in range(DC):
                load_half(wo_t[c], wo_d[128 * c:128 * (c + 1), :])
            bob_t = cpool.tile([128, D], f32, name="bob_t")
            nc.sync.dma_start(bob_t[:], bob_d[:, :])

            # ---- Phase 1a: k projection (no bias: cancels in softmax) ----
            with tc.tile_pool(name="kpp", bufs=2, space="PSUM") as kpp:
                for dp in range(DC):
                    ps = kpp.tile([128, NKV], f32, name="kps")
                    for dc in range(DC):
                        lhs = r(wk_t[dc][:, 128 * dp:128 * (dp + 1)])
                        nc.tensor.matmul(
                            ps[:, 0:512], lhs, r(xkT_t[dc][:, 0:512]),
                            start=(dc == 0), stop=(dc == DC - 1),
                        )
                        nc.tensor.matmul(
                            ps[:, 512:NKV], lhs, r(xkT_t[dc][:, 512:NKV]),
                            start=(dc == 0), stop=(dc == DC - 1),
                        )
                    nc.vector.tensor_scalar_add(r(kT_t[dp][:]), ps[:], 0.0)

            # ---- Phase 1b: q projection ----
            with tc.tile_pool(name="qpp", bufs=2, space="PSUM") as qpp:
                for dp in range(DC):
                    ps = qpp.tile([128, N], f32, name="qps")
                    for dc in range(DC):
                        lhs = r(wq_t[dc][:, 128 * dp:128 * (dp + 1)])
                        for hf in range(2):
                            nc.tensor.matmul(
                                ps[:, 512 * hf:512 * (hf + 1)],
                                lhs,
                                r(xT_t[dc][:, 512 * hf:512 * (hf + 1)]),
                                start=(dc == 0), stop=(dc == DC - 1),
                            )
                    nc.vector.tensor_scalar_add(r(qT_t[dp][:]), ps[:], bq_t[:, dp:dp + 1])

            # ---- Phase 1c: v projection (no bias: folded into bob') ----
            with tc.tile_pool(name="vpp", bufs=2, space="PSUM") as vpp:
                for c in range(KC):
                    ps = vpp.tile([128, H, HD], f32, name="vps")
                    for dc in range(DC):
                        nc.tensor.matmul(
                            ps[:], r(xkT_t[dc][:, 128 * c:128 * (c + 1)]),
                            r(wv_t[dc][:]),
                            start=(dc == 0), stop=(dc == DC - 1),
                        )
                    nc.vector.tensor_scalar_add(r(vaug_t[c][:, :, 0:HD]), ps[:], 0.0)

            # ---- Phase 2: attention on head pairs. Heads 2dp/2dp+1 sit in
            # kT/qT partition rows 0:64 / 64:128, so their K=64 score
            # matmuls land on disjoint PE row tiles (0,0)/(64,0) and run
            # concurrently when issued back-to-back.
            with tc.tile_pool(name="scpA", bufs=1, space="PSUM") as scpA, \
                 tc.tile_pool(name="scpB", bufs=1, space="PSUM") as scpB, \
                 tc.tile_pool(name="oap", bufs=4, space="PSUM") as oap, \
                 tc.tile_pool(name="pp", bufs=6) as pp, \
                 tc.tile_pool(name="rcp", bufs=8) as rcp, \
                 tc.tile_pool(name="rbp", bufs=4) as rbp:
                for dp in range(DC):
                    heads = (2 * dp, 2 * dp + 1)
                    oa = {h: [oap.tile([HD + 1, 512], f32, name="oa")
                              for hf in range(2)] for h in heads}
                    p_t = {h: [] for h in heads}

                    def av(cav):
                        for h in heads:
                            for hf in range(2):
                                nc.tensor.matmul(
                                    oa[h][hf][:],
                                    r(vaug_t[cav][:, h, :]),
                                    r(p_t[h][cav][:, 512 * hf:512 * (hf + 1)]),
                                    start=(cav == 0), stop=(cav == KC - 1),
                                )

                    def norm(h, hf):
                        row = HD * (h % 2)
                        # custom DVE ops read garbage from PSUM on HW:
                        # stage the denominator row through SBUF first
                        db = rcp.tile([1, 512], f32, name="db")
                        nc.vector.tensor_scalar_add(db[:], oa[h][hf][HD:HD + 1, :], 0.0)
                        rc = rcp.tile([1, 512], f32, name="rc")
                        nc.vector.reciprocal_approx_fast(rc[:], db[:])
                        rbs = rbp.tile([HD, 512], f32, name="rbs")
                        nc.gpsimd.partition_broadcast(rbs[:], rc[:])
                        nc.vector.tensor_mul(
                            r(aoT_t[dp][row:row + HD, 512 * hf:512 * (hf + 1)]),
                            oa[h][hf][0:HD, :], rbs[:])

                    for c in range(KC):
                        sc = {heads[0]: scpA.tile([128, N], f32, name="scA"),
                              heads[1]: scpB.tile([128, N], f32, name="scB")}
                        for hf in range(2):
                            for h in heads:
                                row = HD * (h % 2)
                                nc.tensor.matmul(
                                    sc[h][:, 512 * hf:512 * (hf + 1)],
                                    r(kT_t[dp][row:row + HD, 128 * c:128 * (c + 1)]),
                                    r(qT_t[dp][row:row + HD, 512 * hf:512 * (hf + 1)]),
                                    start=True, stop=True,
                                )
                        for h in heads:
                            p = pp.tile([128, N], f32, name="p")
                            nc.scalar.activation(
                                r(p[:]), sc[h][:], AF.Exp,
                                bias=expb_t[:, c:c + 1], scale=SCALE,
                            )
                            p_t[h].append(p)
                        if c >= 1:
                            av(c - 1)
                    av(KC - 1)
                    for h in heads:
                        for hf in range(2):
                            norm(h, hf)

            # ---- Phase 3: output projection ----
            with tc.tile_pool(name="ypp", bufs=2, space="PSUM") as ypp, \
                 tc.tile_pool(name="ysp", bufs=2) as ysp:
                for ic in range(N // 128):
                    yps = ypp.tile([128, D], f32, name="yps")
                    for dp in range(DC):
                        nc.tensor.matmul(
                            yps[:], r(aoT_t[dp][:, 128 * ic:128 * (ic + 1)]),
                            r(wo_t[dp][:]),
                            start=(dp == 0), stop=(dp == DC - 1),
                        )
                    ysb = ysp.tile([128, D], f32, name="ysb")
                    nc.vector.tensor_add(ysb[:], yps[:], bob_t[:])
                    nc.sync.dma_start(y_d[128 * ic:128 * (ic + 1), :], ysb[:])

    return nc


def _get_program():
    if "nc" not in _prog_cache:
        nc = _build_program()
        if not nc.is_finalized():
            nc.finalize()
        _prog_cache["nc"] = nc
    return _prog_cache["nc"]


def _round_fp32r(a):
    # fp32r = fp32 with the mantissa rounded (RNE) to 11 bits (low 12 bits 0)
    bits = np.ascontiguousarray(a, np.float32).view(np.uint32)
    low = bits & np.uint32(0xFFF)
    base = bits & np.uint32(0xFFFFF000)
    lsb = (base >> np.uint32(12)) & np.uint32(1)
    rnd = (low > 0x800) | ((low == 0x800) & (lsb == 1))
    return (base + (rnd.astype(np.uint32) << np.uint32(12))).view(np.float32)


def _prep_core(b, x, mask, wq, bq, wk, bk, wv, bv, wo, bo):
    xb = np.ascontiguousarray(x[b], dtype=np.float32)       # [N, D]
    idx = np.nonzero(mask[b])[0]
    nv = int(idx.size)
    assert 1 <= nv <= NKV, f"batch {b}: {nv} valid keys, NKV={NKV}"
    xk = np.zeros((NKV, D), np.float32)
    xk[:nv] = xb[idx]
    pos = np.arange(128)[:, None] + 128 * np.arange(KC)[None, :]
    expb = np.where(pos < nv, 0.0, PAD_BIAS).astype(np.float32)
    f = np.float32
    bob = (bo.astype(f) + bv.astype(f) @ wo.astype(f)).reshape(D)
    return {
        "xT": _round_fp32r(xb.T),
        "xkT": _round_fp32r(xk.T),
        "wq": _round_fp32r(wq), "wk": _round_fp32r(wk),
        "wv": _round_fp32r(wv), "wo": _round_fp32r(wo),
        "bq": np.ascontiguousarray(bq, f).reshape(D, 1),
        "bob": np.ascontiguousarray(np.broadcast_to(bob, (128, D))),
        "expb": expb,
        "onesv": np.ones((128, H, 1), f),
    }


def _run(inputs):
    import os

    os.environ["BASS_NEVER_TRACE"] = "1"
    from concourse.bass_utils import run_bass_kernel_spmd

    nc = _get_program()
    in_maps = [_prep_core(b, **inputs) for b in range(B)]
    res = run_bass_kernel_spmd(nc, in_maps, core_ids=list(range(B)), trace=False)
    out = np.stack([res.results[b]["y"] for b in range(B)], axis=0)
    return out.astype(np.float32), res


def kernel(**inputs) -> np.ndarray:
    out, _ = _run(inputs)
    return out


# revision 39
# speedup vs baseline: 1.2820x; 1.2820x over previous
"""Multi-head attention (B=8, N=1024, D=512, H=8) on 8 TRN2 NeuronCores.

Sharding: pure batch-parallel — core i computes batch i end-to-end, no
collectives. Host-side prep per batch: gather valid keys (mask) into a
contiguous buffer padded to NKV=640, pre-transpose x, convert streams to
bf16, and build small per-partition bias/validity tables. Device pipeline
(bf16 matmuls, f32 PSUM accumulation):
  k^T/q^T/v projections -> per head pair, scores s^T[k,q] land in one
  [128,1024] PSUM tile (head A cols 0:512 on PE row tile 0, head B cols
  512:1024 on row tile 64, overlapping on disjoint PE rows) -> one exp on
  ACT per query-half with the key-padding mask folded into the activation
  bias -> attn@v with an augmented ones-column producing the softmax
  denominator in row 64 -> fast reciprocal + Pool partition-broadcast ->
  normalize -> out-projection.

Math shortcuts: bk is dropped (constant-in-key terms cancel in softmax);
bv is folded into the output bias on the host (bob' = bo + bv @ wo since
normalized attention rows sum to 1).
"""

import sys

import numpy as np

sys.path.insert(0, "/opt/trn_rl_repo")

B, N, D, H = 8, 1024, 512, 8
HD = D // H            # 64
SCALE = HD ** -0.5     # 0.125
NKV = 640              # padded valid-key count (5 chunks of 128)
KC = NKV // 128        # 5
DC = D // 128          # 4
VW = HD + 2            # 66: aug head stride, 4B-aligned for bf16 weights
PAD_BIAS = -30000.0    # exp(PAD_BIAS + s*SCALE) == 0.0 exactly

_prog_cache = {}


def _build_program():
    import concourse.bacc as bacc
    import concourse.tile as tile
    from concourse import mybir

    dt = mybir.dt
    f32 = dt.float32
    bf16 = dt.bfloat16
    AF = mybir.ActivationFunctionType

    nc = bacc.Bacc("TRN2", target_bir_lowering=False, debug=False)

    xT_d = nc.dram_tensor("xT", [D, N], bf16, kind="ExternalInput").ap()
    xkT_d = nc.dram_tensor("xkT", [D, NKV], bf16, kind="ExternalInput").ap()
    wq_d = nc.dram_tensor("wq", [D, D], bf16, kind="ExternalInput").ap()
    wk_d = nc.dram_tensor("wk", [D, D], bf16, kind="ExternalInput").ap()
    wv_d = nc.dram_tensor("wv", [D, D], bf16, kind="ExternalInput").ap()
    wo_d = nc.dram_tensor("wo", [D, D], bf16, kind="ExternalInput").ap()
    bq_d = nc.dram_tensor("bq", [D, 1], f32, kind="ExternalInput").ap()
    bob_d = nc.dram_tensor("bob", [128, D], f32, kind="ExternalInput").ap()
    expb_d = nc.dram_tensor("expb", [128, KC], f32, kind="ExternalInput").ap()
    onesv_d = nc.dram_tensor("onesv", [128, H, 1], bf16,
                             kind="ExternalInput").ap()
    y_d = nc.dram_tensor("y", [N, D], f32, kind="ExternalOutput").ap()

    with tile.TileContext(nc) as tc, \
         nc.allow_low_precision(reason="bf16 matmul streams, f32 accumulate"):
        with tc.tile_pool(name="const", bufs=1) as cpool:
            # Persistent result tiles (vaug gets its ones column via DMA)
            qT_t = [cpool.tile([128, N], bf16, name=f"qT_t{c}")
                    for c in range(DC)]
            kT_t = [cpool.tile([128, NKV], bf16, name=f"kT_t{c}")
                    for c in range(DC)]
            vaug_t = [cpool.tile([128, H, VW], bf16, name=f"vaug_t{c}")
                      for c in range(KC)]
            aoT_t = [cpool.tile([128, N], bf16, name=f"aoT_t{c}")
                     for c in range(DC)]

            # DMA issue order = priority order (queues drain round-robin).
            # k-projection inputs first (PE starts on them), tiny tables,
            # then q inputs, v, and wo/bob last. Big loads split into
            # partition halves so they spread across two queues.
            def load_half(t, dram_slice):
                nc.sync.dma_start(t[0:64, :], dram_slice[0:64, :])
                nc.sync.dma_start(t[64:128, :], dram_slice[64:128, :])

            def alloc_w(name):
                return [cpool.tile([128, D], bf16, name=f"{name}{c}")
                        for c in range(DC)]

            wk_t, wq_t, wv_t, wo_t = (alloc_w(n) for n in
                                      ("wk_t", "wq_t", "wv_t", "wo_t"))
            xkT_t = [cpool.tile([128, NKV], bf16, name=f"xkT_t{c}")
                     for c in range(DC)]
            xT_t = [cpool.tile([128, N], bf16, name=f"xT_t{c}")
                    for c in range(DC)]

            for c in range(DC):
                load_half(wk_t[c], wk_d[128 * c:128 * (c + 1), :])
                load_half(xkT_t[c], xkT_d[128 * c:128 * (c + 1), :])
            bq_t = cpool.tile([128, DC], f32, name="bq_t")
            for c in range(DC):
                nc.sync.dma_start(bq_t[:, c:c + 1],
                                  bq_d[128 * c:128 * (c + 1), :])
            expb_t = cpool.tile([128, KC], f32, name="expb_t")
            nc.sync.dma_start(expb_t[:], expb_d[:, :])
            for c in range(KC):
                nc.sync.dma_start(vaug_t[c][:, :, HD:HD + 1], onesv_d[:])
            for c in range(DC):
                load_half(wq_t[c], wq_d[128 * c:128 * (c + 1), :])
                load_half(xT_t[c], xT_d[128 * c:128 * (c + 1), :])
            for c in range(DC):
                load_half(wv_t[c], wv_d[128 * c:128 * (c + 1), :])
            for c in range(DC):
                load_half(wo_t[c], wo_d[128 * c:128 * (c + 1), :])
            bob_t = cpool.tile([128, D], f32, name="bob_t")
            nc.sync.dma_start(bob_t[:], bob_d[:, :])

            # ---- Phase 1a: k projection (no bias: cancels in softmax) ----
            with tc.tile_pool(name="kpp", bufs=2, space="PSUM") as kpp:
                for dp in range(DC):
                    ps = kpp.tile([128, NKV], f32, name="kps")
                    for dc in range(DC):
                        lhs = wk_t[dc][:, 128 * dp:128 * (dp + 1)]
                        nc.tensor.matmul(
                            ps[:, 0:512], lhs, xkT_t[dc][:, 0:512],
                            start=(dc == 0), stop=(dc == DC - 1),
                        )
                        nc.tensor.matmul(
                            ps[:, 512:NKV], lhs, xkT_t[dc][:, 512:NKV],
                            start=(dc == 0), stop=(dc == DC - 1),
                        )
                    nc.vector.tensor_scalar_add(kT_t[dp][:], ps[:], 0.0)

            # ---- Phase 1b: q projection ----
            with tc.tile_pool(name="qpp", bufs=2, space="PSUM") as qpp:
                for dp in range(DC):
                    ps = qpp.tile([128, N], f32, name="qps")
                    for dc in range(DC):
                        lhs = wq_t[dc][:, 128 * dp:128 * (dp + 1)]
                        for hf in range(2):
                            nc.tensor.matmul(
                                ps[:, 512 * hf:512 * (hf + 1)],
                                lhs,
                                xT_t[dc][:, 512 * hf:512 * (hf + 1)],
                                start=(dc == 0), stop=(dc == DC - 1),
                            )
                    nc.vector.tensor_scalar_add(qT_t[dp][:], ps[:],
                                                bq_t[:, dp:dp + 1])

            # ---- Phase 1c: v projection (no bias: folded into bob') ----
            with tc.tile_pool(name="vpp", bufs=2, space="PSUM") as vpp:
                for c in range(KC):
                    ps = vpp.tile([128, H, HD], f32, name="vps")
                    for dc in range(DC):
                        nc.tensor.matmul(
                            ps[:], xkT_t[dc][:, 128 * c:128 * (c + 1)],
                            wv_t[dc][:],
                            start=(dc == 0), stop=(dc == DC - 1),
                        )
                    nc.vector.tensor_scalar_add(vaug_t[c][:, :, 0:HD],
                                                ps[:], 0.0)

            # ---- Phase 2: attention on head pairs. Heads A=2dp (kT/qT
            # rows 0:64) and B=2dp+1 (rows 64:128) write one [128,1024]
            # score tile per query-half; the shared exp makes both matmuls
            # feed one consumer so they stay adjacent in the PE stream and
            # overlap on disjoint PE row tiles (0,0)/(64,0).
            with tc.tile_pool(name="scp", bufs=2, space="PSUM") as scp, \
                 tc.tile_pool(name="oap", bufs=4, space="PSUM") as oap, \
                 tc.tile_pool(name="pp", bufs=6) as pp, \
                 tc.tile_pool(name="rcp", bufs=6) as rcp:
                for dp in range(DC):
                    heads = (2 * dp, 2 * dp + 1)
                    oa = {h: [oap.tile([HD + 1, 512], f32, name="oa")
                              for hf in range(2)] for h in heads}
                    p_t = []  # per chunk: [p_hf0, p_hf1]

                    def av(cav):
                        for hf in range(2):
                            for hi, h in enumerate(heads):
                                nc.tensor.matmul(
                                    oa[h][hf][:],
                                    vaug_t[cav][:, h, 0:HD + 1],
                                    p_t[cav][hf][:, 512 * hi:512 * (hi + 1)],
                                    start=(cav == 0), stop=(cav == KC - 1),
                                )

                    for c in range(KC):
                        ps = []
                        for hf in range(2):
                            sc = scp.tile([128, N], f32, name="sc")
                            for hi, h in enumerate(heads):
                                row = HD * (h % 2)
                                nc.tensor.matmul(
                                    sc[:, 512 * hi:512 * (hi + 1)],
                                    kT_t[dp][row:row + HD,
                                             128 * c:128 * (c + 1)],
                                    qT_t[dp][row:row + HD,
                                             512 * hf:512 * (hf + 1)],
                                    start=True, stop=True,
                                )
                            p = pp.tile([128, N], bf16, name="p")
                            nc.scalar.activation(
                                p[:], sc[:], AF.Exp,
                                bias=expb_t[:, c:c + 1], scale=SCALE,
                            )
                            ps.append(p)
                        p_t.append(ps)
                        if c >= 1:
                            av(c - 1)
                    av(KC - 1)

                    for h in heads:
                        for hf in range(2):
                            row = HD * (h % 2)
                            # custom DVE ops read garbage from PSUM on HW:
                            # stage the denominator row through SBUF first
                            db = rcp.tile([1, 512], f32, name="db")
                            nc.vector.tensor_scalar_add(
                                db[:], oa[h][hf][HD:HD + 1, :], 0.0)
                            rc = rcp.tile([1, 512], f32, name="rc")
                            nc.vector.reciprocal_approx_fast(rc[:], db[:])
                            rbs = rcp.tile([HD, 512], f32, name="rbs")
                            nc.gpsimd.partition_broadcast(rbs[:], rc[:])
                            nc.vector.tensor_mul(
                                aoT_t[dp][row:row + HD,
                                          512 * hf:512 * (hf + 1)],
                                oa[h][hf][0:HD, :], rbs[:])

            # ---- Phase 3: output projection ----
            with tc.tile_pool(name="ypp", bufs=2, space="PSUM") as ypp, \
                 tc.tile_pool(name="ysp", bufs=2) as ysp:
                for ic in range(N // 128):
                    yps = ypp.tile([128, D], f32, name="yps")
                    for dp in range(DC):
                        nc.tensor.matmul(
                            yps[:], aoT_t[dp][:, 128 * ic:128 * (ic + 1)],
                            wo_t[dp][:],
                            start=(dp == 0), stop=(dp == DC - 1),
                        )
                    ysb = ysp.tile([128, D], f32, name="ysb")
                    nc.vector.tensor_add(ysb[:], yps[:], bob_t[:])
                    nc.sync.dma_start(y_d[128 * ic:128 * (ic + 1), :], ysb[:])

    return nc


def _get_program():
    if "nc" not in _prog_cache:
        nc = _build_program()
        if not nc.is_finalized():
            nc.finalize()
        _prog_cache["nc"] = nc
    return _prog_cache["nc"]


def _prep_core(b, x, mask, wq, bq, wk, bk, wv, bv, wo, bo):
    import ml_dtypes

    b16 = ml_dtypes.bfloat16
    xb = np.ascontiguousarray(x[b], dtype=np.float32)       # [N, D]
    idx = np.nonzero(mask[b])[0]
    nv = int(idx.size)
    assert 1 <= nv <= NKV, f"batch {b}: {nv} valid keys, NKV={NKV}"
    xk = np.zeros((NKV, D), np.float32)
    xk[:nv] = xb[idx]
    pos = np.arange(128)[:, None] + 128 * np.arange(KC)[None, :]
    expb = np.where(pos < nv, 0.0, PAD_BIAS).astype(np.float32)
    f = np.float32
    bob = (bo.astype(f) + bv.astype(f) @ wo.astype(f)).reshape(D)
    return {
        "xT": np.ascontiguousarray(xb.T).astype(b16),
        "xkT": np.ascontiguousarray(xk.T).astype(b16),
        "wq": np.ascontiguousarray(wq, f).astype(b16),
        "wk": np.ascontiguousarray(wk, f).astype(b16),
        "wv": np.ascontiguousarray(wv, f).astype(b16),
        "wo": np.ascontiguousarray(wo, f).astype(b16),
        "bq": np.ascontiguousarray(bq, f).reshape(D, 1),
        "bob": np.ascontiguousarray(np.broadcast_to(bob, (128, D))),
        "expb": expb,
        "onesv": np.ones((128, H, 1), b16),
    }


def _run(inputs):
    import os

    os.environ["BASS_NEVER_TRACE"] = "1"
    from concourse.bass_utils import run_bass_kernel_spmd

    nc = _get_program()
    in_maps = [_prep_core(b, **inputs) for b in range(B)]
    res = run_bass_kernel_spmd(nc, in_maps, core_ids=list(range(B)),
                               trace=False)
    out = np.stack([res.results[b]["y"] for b in range(B)], axis=0)
    return out.astype(np.float32), res


def kernel(**inputs) -> np.ndarray:
    out, _ = _run(inputs)
    return out
